# revision 4
# baseline (speedup 1.0000x reference)
"""Trainium2 Bass kernel for a DoReFa-quantized ResNet BasicBlock.

    out = act(bn2(conv3x3(act(bn1(conv3x3(x, qw(w1)))), qw(w2))) + x)

with 4-bit DoReFa weight/activation quantization and training-mode BatchNorm
(batch statistics over N,H,W).

Strategy (8 NeuronCores, data-parallel over batch):
 - batch N=64 sharded 8 images/core; weights replicated.
 - BN uses *synced* batch statistics: per-core per-channel sum/sumsq are
   AllReduced across the 8 cores (two tiny [128,2] collectives).
 - conv3x3 = 9 shifted matmuls accumulated in PSUM (C_in on partitions,
   pixels on free dim), using a zero-padded [C,58,58] SBUF image.
 - Weight quantization produces small odd integers (2m-15, |.| <= 15) which
   are exact in bf16/fp8; the /15 scales are folded into the BN affine maps.
   conv1 runs in bf16 with a hi/lo split of x (x = hi + lo, each bf16) for
   ~f32 accuracy; conv2 inputs are exact small ints and run in fp8 exactly.
 - Activation quantization uses the 2^23 magic-constant round-to-nearest-even
   (matches jnp.round) after clipping via Relu/min.
"""

import numpy as np

import concourse.bacc as bacc
import concourse.mybir as mybir
import concourse.tile as tile
import concourse.bass_isa as bass_isa
from concourse.bass_utils import run_bass_kernel_spmd
from concourse.bass_interp import get_hw_module
from concourse.masks import make_identity

F32 = mybir.dt.float32
BF16 = mybir.dt.bfloat16
FP8 = mybir.dt.float8e4
AF = mybir.ActivationFunctionType
ALU = mybir.AluOpType

N_CORES = 8
N_PER = 8            # images per core
C = 128              # channels
H = W = 56
HW = H * W           # 3136
PW = 58              # padded height/width
RCH = 8              # output rows per chunk
NCHUNK = H // RCH    # 7 chunks per image
CHN = RCH * W        # 448 pixels per chunk
MAGIC = float(2.0 ** 23)
N_BATCH = 64 * HW    # BN sample count (full batch)
INV_N = float(np.float32(1.0 / N_BATCH))
EPS1 = float(np.float32(225e-5))      # 15^2 * 1e-5   (conv1 output scale)
EPS2 = float(np.float32(50625e-5))    # 225^2 * 1e-5  (conv2 output scale)
INV15 = float(np.float32(1.0 / 15.0))
# tanh(w) = w * (1 + w2*(c1 + w2*(c2 + w2*(c3 + w2*c4)))), |w| < 0.25
TC1 = float(np.float32(-1.0 / 3.0))
TC2 = float(np.float32(2.0 / 15.0))
TC3 = float(np.float32(-17.0 / 315.0))
TC4 = float(np.float32(62.0 / 2835.0))

_CACHED = {}


def _quant_weights(nc, tc, pool_T, consts, ptr, ident, w_ap, lhsT, name):
    """DoReFa-quantize a [128,128,3,3] weight into per-tap transposed integer
    weight tiles lhsT[c_in, tap, c_out] holding exact odd ints 2m-15."""
    wsb = pool_T.tile([C, C * 9], F32, tag="T")
    nc.sync.dma_start(wsb[:], w_ap.rearrange("o i kh kw -> o (i kh kw)"))
    w2 = pool_T.tile([C, C * 9], F32, tag="T")
    nc.vector.tensor_tensor(w2[:], wsb[:], wsb[:], ALU.mult)
    # Horner for tanh polynomial (f32)
    p = pool_T.tile([C, C * 9], F32, tag="T")
    nc.vector.tensor_scalar(p[:], w2[:], TC4, TC3, ALU.mult, ALU.add)
    nc.vector.tensor_tensor(p[:], p[:], w2[:], ALU.mult)
    nc.vector.tensor_scalar(p[:], p[:], TC2, None, ALU.add)
    nc.vector.tensor_tensor(p[:], p[:], w2[:], ALU.mult)
    nc.vector.tensor_scalar(p[:], p[:], TC1, None, ALU.add)
    nc.vector.tensor_tensor(p[:], p[:], w2[:], ALU.mult)
    nc.vector.tensor_scalar(p[:], p[:], 1.0, None, ALU.add)
    wt = pool_T.tile([C, C * 9], F32, tag="T")
    nc.vector.tensor_tensor(wt[:], wsb[:], p[:], ALU.mult)
    # global max |tanh(w)|
    amax = consts.tile([C, 1], F32, tag=f"amax{name}")
    nc.vector.tensor_reduce(amax[:], wt[:], mybir.AxisListType.X, ALU.max,
                            apply_absolute_value=True)
    gmax = consts.tile([C, 1], F32, tag=f"gmax{name}")
    nc.gpsimd.partition_all_reduce(gmax[:], amax[:], C, bass_isa.ReduceOp.max)
    inv2m = consts.tile([C, 1], F32, tag=f"inv2m{name}")
    nc.vector.tensor_scalar(inv2m[:], gmax[:], 2.0, None, ALU.mult)
    nc.vector.reciprocal(inv2m[:], inv2m[:])
    # codes m = round((wt/(2M) + 0.5) * 15); Wi = 2m - 15 (exact ints)
    wn = pool_T.tile([C, C * 9], F32, tag="T")
    nc.vector.tensor_scalar(wn[:], wt[:], inv2m[:, 0:1], None, ALU.mult)
    nc.vector.tensor_scalar(wn[:], wn[:], 0.5, 15.0, ALU.add, ALU.mult)
    nc.vector.tensor_scalar(wn[:], wn[:], MAGIC, -MAGIC, ALU.add, ALU.add)
    wi = pool_T.tile([C, C * 9], BF16, tag="T")
    nc.vector.tensor_scalar(wi[:], wn[:], 2.0, -15.0, ALU.mult, ALU.add)
    # transpose each tap via PE: lhsT[i, t, o] = Wi[o, i*9+t]
    wir = wi.rearrange("o (i t) -> o i t", t=9)
    for t in range(9):
        pst = ptr.tile([C, C], BF16, tag="tr")
        nc.tensor.transpose(pst[:], wir[:, :, t], ident[:])
        nc.scalar.copy(lhsT[:, t, :], pst[:])


def _bn_vectors(nc, consts, rstats, gamma_ap, beta_ap, eps, post_scale, name):
    """From allreduced [sum, sumsq] and gamma/beta, build per-channel
    scale/bias [128,1] tiles s.t. y = T*scale + bias equals
    post_scale * batchnorm(T/k).  eps is pre-scaled by k^2."""
    g = consts.tile([C, 1], F32, tag=f"g{name}")
    nc.sync.dma_start(g[:], gamma_ap.rearrange("(c one) -> c one", one=1))
    b = consts.tile([C, 1], F32, tag=f"b{name}")
    nc.sync.dma_start(b[:], beta_ap.rearrange("(c one) -> c one", one=1))
    mean = consts.tile([C, 1], F32, tag=f"mean{name}")
    nc.vector.tensor_scalar(mean[:], rstats[:, 0:1], INV_N, None, ALU.mult)
    var = consts.tile([C, 1], F32, tag=f"var{name}")
    nc.vector.tensor_scalar(var[:], rstats[:, 1:2], INV_N, None, ALU.mult)
    msq = consts.tile([C, 1], F32, tag=f"msq{name}")
    nc.vector.tensor_tensor(msq[:], mean[:], mean[:], ALU.mult)
    nc.vector.tensor_tensor(var[:], var[:], msq[:], ALU.subtract)
    epst = consts.tile([C, 1], F32, tag=f"eps{name}")
    nc.vector.memset(epst[:], eps)
    std = consts.tile([C, 1], F32, tag=f"std{name}")
    nc.scalar.activation(std[:], var[:], AF.Sqrt, bias=epst[:, 0:1], scale=1.0)
    inv = consts.tile([C, 1], F32, tag=f"inv{name}")
    nc.vector.reciprocal(inv[:], std[:])
    scale = consts.tile([C, 1], F32, tag=f"scale{name}")
    nc.vector.tensor_tensor(scale[:], g[:], inv[:], ALU.mult)
    if post_scale != 1.0:
        nc.vector.tensor_scalar(scale[:], scale[:], post_scale, None, ALU.mult)
    bias = consts.tile([C, 1], F32, tag=f"bias{name}")
    nc.vector.tensor_tensor(bias[:], mean[:], scale[:], ALU.mult)
    if post_scale != 1.0:
        nc.vector.tensor_scalar(b[:], b[:], post_scale, None, ALU.mult)
    nc.vector.tensor_tensor(bias[:], b[:], bias[:], ALU.subtract)
    return scale, bias


def build():
    nc = bacc.Bacc("TRN2", target_bir_lowering=False, debug=False,
                   num_devices=N_CORES)
    x_ap = nc.dram_tensor("x", [N_PER, C, H, W], F32, kind="ExternalInput").ap()
    w1_ap = nc.dram_tensor("w1", [C, C, 3, 3], F32, kind="ExternalInput").ap()
    w2_ap = nc.dram_tensor("w2", [C, C, 3, 3], F32, kind="ExternalInput").ap()
    g1_ap = nc.dram_tensor("gamma1", [C], F32, kind="ExternalInput").ap()
    b1_ap = nc.dram_tensor("beta1", [C], F32, kind="ExternalInput").ap()
    g2_ap = nc.dram_tensor("gamma2", [C], F32, kind="ExternalInput").ap()
    b2_ap = nc.dram_tensor("beta2", [C], F32, kind="ExternalInput").ap()
    out_ap = nc.dram_tensor("out", [N_PER, C, H, W], F32,
                            kind="ExternalOutput").ap()
    x_r = x_ap.rearrange("n c h w -> n c h w")
    out_r = out_ap.rearrange("n c h w -> n c h w")

    with tile.TileContext(nc) as tc:
        with tc.tile_pool(name="consts", bufs=1) as consts, \
             tc.tile_pool(name="T", bufs=N_PER) as pool_T, \
             tc.tile_pool(name="padhl", bufs=4) as padhl, \
             tc.tile_pool(name="apad", bufs=N_PER) as apadp, \
             tc.tile_pool(name="xio", bufs=3) as xio, \
             tc.tile_pool(name="xres", bufs=3) as xres, \
             tc.tile_pool(name="outs", bufs=2) as outsp, \
             tc.tile_pool(name="psum", bufs=4, space="PSUM") as psum, \
             tc.tile_pool(name="ptr", bufs=1, space="PSUM") as ptr, \
             tc.tile_pool(name="psq", bufs=2, space="PSUM") as psq, \
             tc.tile_pool(name="stats", bufs=1) as statsp, \
             tc.tile_pool(name="dram", bufs=1, space="DRAM") as dram:

            ident = consts.tile([C, C], BF16, tag="ident")
            make_identity(nc, ident[:])
            lhsT1 = consts.tile([C, 9, C], BF16, tag="lhsT1")
            lhsT2 = consts.tile([C, 9, C], FP8, tag="lhsT2")
            _quant_weights(nc, tc, pool_T, consts, ptr, ident, w1_ap, lhsT1, "1")
            _quant_weights(nc, tc, pool_T, consts, ptr, ident, w2_ap, lhsT2, "2")

            # ---------------- phase 1: conv1 + stats ----------------
            s1 = statsp.tile([C, N_PER * NCHUNK], F32, tag="s1")
            q1 = statsp.tile([C, N_PER * NCHUNK], F32, tag="q1")
            T1 = []
            for i in range(N_PER):
                xph = padhl.tile([C, PW, PW], BF16, tag="pad")
                xpl = padhl.tile([C, PW, PW], BF16, tag="pad")
                nc.gpsimd.memset(xph[:], 0.0)
                nc.gpsimd.memset(xpl[:], 0.0)
                # load x in 4 row-groups of 14, split hi/lo into padded tiles
                for g in range(4):
                    xs = xio.tile([C, 14, W], F32, tag="xio")
                    nc.sync.dma_start(xs[:], x_r[i, :, g * 14:(g + 1) * 14, :])
                    dst_h = xph[:, 1 + g * 14:1 + (g + 1) * 14, 1:57]
                    dst_l = xpl[:, 1 + g * 14:1 + (g + 1) * 14, 1:57]
                    nc.scalar.copy(dst_h, xs[:])
                    nc.vector.tensor_tensor(dst_l, xs[:], dst_h, ALU.subtract)
                Ti = pool_T.tile([C, HW], F32, tag="T")
                T1.append(Ti)
                Tir = Ti.rearrange("c (h w) -> c h w", w=W)
                for ck in range(NCHUNK):
                    ps = psum.tile([C, CHN], F32, tag="mm")
                    r0 = ck * RCH
                    k = 0
                    for ky in range(3):
                        for kx in range(3):
                            rhs_h = xph[:, r0 + ky:r0 + ky + RCH, kx:kx + W]
                            rhs_l = xpl[:, r0 + ky:r0 + ky + RCH, kx:kx + W]
                            nc.tensor.matmul(ps[:], lhsT1[:, 3 * ky + kx, :],
                                             rhs_h, start=(k == 0), stop=False)
                            nc.tensor.matmul(ps[:], lhsT1[:, 3 * ky + kx, :],
                                             rhs_l, start=False, stop=(k == 8))
                            k += 1
                    col = i * NCHUNK + ck
                    nc.scalar.activation(Tir[:, r0:r0 + RCH, :], ps[:], AF.Copy,
                                         accum_out=s1[:, col:col + 1])
                    sq = psq.tile([C, CHN], F32, tag="sq")
                    nc.scalar.activation(sq[:], ps[:], AF.Square,
                                         accum_out=q1[:, col:col + 1])

            # allreduce stats 1
            st1 = statsp.tile([C, 2], F32, tag="st1")
            nc.vector.tensor_reduce(st1[:, 0:1], s1[:], mybir.AxisListType.X,
                                    ALU.add)
            nc.vector.tensor_reduce(st1[:, 1:2], q1[:], mybir.AxisListType.X,
                                    ALU.add)
            cc1i = dram.tile([C, 2], F32, tag="cc1i")
            cc1o = dram.tile([C, 2], F32, tag="cc1o")
            nc.gpsimd.dma_start(cc1i[:], st1[:])
            nc.gpsimd.collective_compute(
                "AllReduce", ALU.add, replica_groups=[list(range(N_CORES))],
                ins=[cc1i.opt()], outs=[cc1o.opt()])
            rst1 = statsp.tile([C, 2], F32, tag="rst1")
            nc.gpsimd.dma_start(rst1[:], cc1o[:])
            sc1, bi1 = _bn_vectors(nc, consts, rst1, g1_ap, b1_ap, EPS1, 15.0, "1")

            # ---------------- phase 2: act1 + conv2 + stats ----------------
            s2 = statsp.tile([C, N_PER * NCHUNK], F32, tag="s2")
            q2 = statsp.tile([C, N_PER * NCHUNK], F32, tag="q2")
            T2 = []
            apads = []
            for i in range(N_PER):
                ap_t = apadp.tile([C, PW, PW], FP8, tag="apad")
                apads.append(ap_t)
                nc.gpsimd.memset(ap_t[:], 0.0)
                Tir = T1[i].rearrange("c (h w) -> c h w", w=W)
                for ck in range(NCHUNK):
                    r0 = ck * RCH
                    y15 = outsp.tile([C, RCH, W], F32, tag="y15")
                    nc.scalar.activation(y15[:], Tir[:, r0:r0 + RCH, :], AF.Relu,
                                         bias=bi1[:, 0:1], scale=sc1[:, 0:1])
                    nc.vector.tensor_scalar(y15[:], y15[:], 15.0, MAGIC,
                                            ALU.min, ALU.add)
                    nc.vector.tensor_scalar(
                        ap_t[:, 1 + r0:1 + r0 + RCH, 1:57], y15[:],
                        -MAGIC, None, ALU.add)
            for i in range(N_PER):
                ap_t = apads[i]
                Ti2 = pool_T.tile([C, HW], F32, tag="T")
                T2.append(Ti2)
                T2r = Ti2.rearrange("c (h w) -> c h w", w=W)
                for ck in range(NCHUNK):
                    ps = psum.tile([C, CHN], F32, tag="mm")
                    r0 = ck * RCH
                    for k, (ky, kx) in enumerate(
                            (a, b) for a in range(3) for b in range(3)):
                        rhs = ap_t[:, r0 + ky:r0 + ky + RCH, kx:kx + W]
                        nc.tensor.matmul(ps[:], lhsT2[:, 3 * ky + kx, :], rhs,
                                         start=(k == 0), stop=(k == 8))
                    col = i * NCHUNK + ck
                    nc.scalar.activation(T2r[:, r0:r0 + RCH, :], ps[:], AF.Copy,
                                         accum_out=s2[:, col:col + 1])
                    sq = psq.tile([C, CHN], F32, tag="sq")
                    nc.scalar.activation(sq[:], ps[:], AF.Square,
                                         accum_out=q2[:, col:col + 1])

            # allreduce stats 2
            st2 = statsp.tile([C, 2], F32, tag="st2")
            nc.vector.tensor_reduce(st2[:, 0:1], s2[:], mybir.AxisListType.X,
                                    ALU.add)
            nc.vector.tensor_reduce(st2[:, 1:2], q2[:], mybir.AxisListType.X,
                                    ALU.add)
            cc2i = dram.tile([C, 2], F32, tag="cc2i")
            cc2o = dram.tile([C, 2], F32, tag="cc2o")
            nc.gpsimd.dma_start(cc2i[:], st2[:])
            nc.gpsimd.collective_compute(
                "AllReduce", ALU.add, replica_groups=[list(range(N_CORES))],
                ins=[cc2i.opt()], outs=[cc2o.opt()])
            rst2 = statsp.tile([C, 2], F32, tag="rst2")
            nc.gpsimd.dma_start(rst2[:], cc2o[:])
            sc2, bi2 = _bn_vectors(nc, consts, rst2, g2_ap, b2_ap, EPS2, 1.0, "2")

            # ---------------- phase 3: bn2 + residual + act ----------------
            for i in range(N_PER):
                T2r = T2[i].rearrange("c (h w) -> c h w", w=W)
                for ck in range(NCHUNK):
                    r0 = ck * RCH
                    xr = xres.tile([C, RCH, W], F32, tag="xres")
                    nc.sync.dma_start(xr[:], x_r[i, :, r0:r0 + RCH, :])
                    y2 = outsp.tile([C, RCH, W], F32, tag="y2")
                    nc.scalar.activation(y2[:], T2r[:, r0:r0 + RCH, :],
                                         AF.Identity, bias=bi2[:, 0:1],
                                         scale=sc2[:, 0:1])
                    nc.vector.tensor_tensor(y2[:], y2[:], xr[:], ALU.add)
                    z = outsp.tile([C, RCH, W], F32, tag="z")
                    nc.scalar.activation(z[:], y2[:], AF.Relu, bias=0.0,
                                         scale=15.0)
                    nc.vector.tensor_scalar(z[:], z[:], 15.0, MAGIC,
                                            ALU.min, ALU.add)
                    nc.vector.tensor_scalar(z[:], z[:], -MAGIC, INV15,
                                            ALU.add, ALU.mult)
                    nc.sync.dma_start(out_r[i, :, r0:r0 + RCH, :], z[:])

    nc.compile()
    return nc


def kernel(x, w1, w2, gamma1, beta1, gamma2, beta2):
    if "nc" not in _CACHED:
        _CACHED["nc"] = build()
    nc = _CACHED["nc"]
    x = np.ascontiguousarray(x, dtype=np.float32)
    shard = x.reshape(N_CORES, N_PER, C, H, W)
    common = {
        "w1": np.ascontiguousarray(w1, np.float32),
        "w2": np.ascontiguousarray(w2, np.float32),
        "gamma1": np.ascontiguousarray(gamma1, np.float32),
        "beta1": np.ascontiguousarray(beta1, np.float32),
        "gamma2": np.ascontiguousarray(gamma2, np.float32),
        "beta2": np.ascontiguousarray(beta2, np.float32),
    }
    in_maps = [{"x": shard[i], **common} for i in range(N_CORES)]
    old_m = nc.m
    nc.m = get_hw_module(nc.m)
    try:
        res = run_bass_kernel_spmd(nc, in_maps, core_ids=list(range(N_CORES)))
    finally:
        nc.m = old_m
    out = np.concatenate([res.results[i]["out"] for i in range(N_CORES)], axis=0)
    return out.astype(np.float32)
